# revision 1
# baseline (speedup 1.0000x reference)
"""Bass/Trainium2 kernel for nn_Inplace4pHermiteResampler.

Strategy (8 NeuronCores, output-sample sharded):
  reference: out[c,j] = ((c3*x+c2)*x+c1)*x + y0v  with taps gathered at
  ym1/y0/y1/y2 indices. Algebraically identical 4-tap FIR:
      out[c,j] = sum_t w_t(x[j]) * y[c, idx_t[j]]
  Host (numpy): computes the 4 weight vectors w_t(x) in f64->f32 and
  materializes the 4 gathered tap streams U_t[c,j] = y[c, idx_t[j]]
  (pure data movement), shards j across 8 cores, and lays everything out
  as contiguous [128, 2*941] tiles.
  Device (Bass/Tile): per 2-channel group, 4 TT multiplies + 3 TT adds on
  the Vector engine, DMA in/out double-buffered. Weights are channel-shared
  and loaded once (replicated x2 to keep every AP contiguous).
"""
import os

os.environ.setdefault("NEURON_RT_VIRTUAL_CORE_SIZE", "1")

import numpy as np

N_CH = 32
N_IN = 1_048_576
N_OUT = 963_380
N_CORES = 8
F = 941                      # free-dim cols per partition row
P = 128
JPAD = P * F                 # 120448 padded j per core
NG = N_CH // 2               # 16 two-channel groups

_STARTS = [(N_OUT * i) // N_CORES for i in range(N_CORES + 1)]


def _hermite_weights(x64: np.ndarray) -> np.ndarray:
    """4 Niemitalo weights per output sample, [4, n] float32."""
    x2 = x64 * x64
    x3 = x2 * x64
    return np.stack(
        [
            -0.5 * x3 + x2 - 0.5 * x64,
            1.5 * x3 - 2.5 * x2 + 1.0,
            -1.5 * x3 + 2.0 * x2 + 0.5 * x64,
            0.5 * x3 - 0.5 * x2,
        ],
        0,
    ).astype(np.float32)


def _build_device_kernel():
    import concourse.bacc as bacc
    import concourse.mybir as mybir
    import concourse.tile as tile

    nc = bacc.Bacc(
        "TRN2",
        target_bir_lowering=False,
        debug=False,
        enable_asserts=False,
        num_devices=N_CORES,
    )
    dt = mybir.dt.float32
    u_d = nc.dram_tensor("u", [4, NG, P, 2 * F], dt, kind="ExternalInput").ap()
    w_d = nc.dram_tensor("w", [4, P, 2 * F], dt, kind="ExternalInput").ap()
    o_d = nc.dram_tensor("o", [NG, P, 2 * F], dt, kind="ExternalOutput").ap()

    mult = mybir.AluOpType.mult
    add = mybir.AluOpType.add

    with tile.TileContext(nc) as tc:
        with (
            tc.tile_pool(name="wp", bufs=1) as wp,
            tc.tile_pool(name="up", bufs=10) as up,
            tc.tile_pool(name="ap", bufs=3) as apool,
            tc.tile_pool(name="qp", bufs=3) as qp,
        ):
            wt = []
            for t in range(4):
                w_tile = wp.tile([P, 2 * F], dt, tag=f"w{t}")
                nc.sync.dma_start(out=w_tile[:], in_=w_d[t])
                wt.append(w_tile)
            for g in range(NG):
                ut = []
                for t in range(4):
                    u_tile = up.tile([P, 2 * F], dt, tag="u")
                    # spread loads across both HWDGE engines (SP + ACT)
                    eng = nc.sync if t % 2 == 0 else nc.scalar
                    eng.dma_start(out=u_tile[:], in_=u_d[t, g])
                    ut.append(u_tile)
                acc = apool.tile([P, 2 * F], dt, tag="acc")
                nc.vector.tensor_tensor(
                    out=acc[:], in0=ut[0][:], in1=wt[0][:], op=mult
                )
                for t in range(1, 4):
                    q = qp.tile([P, 2 * F], dt, tag="q")
                    nc.vector.tensor_tensor(
                        out=q[:], in0=ut[t][:], in1=wt[t][:], op=mult
                    )
                    nc.vector.tensor_tensor(
                        out=acc[:], in0=acc[:], in1=q[:], op=add
                    )
                nc.sync.dma_start(out=o_d[g], in_=acc[:])
    nc.compile()
    return nc


_NC_CACHE = None


def _get_nc():
    global _NC_CACHE
    if _NC_CACHE is None:
        _NC_CACHE = _build_device_kernel()
    return _NC_CACHE


def _prep_inputs(y, x, y_m1_idx, y0_idx, y1_idx, y2_idx):
    """Host-side shard + restructure. Returns per-core in_maps."""
    y = np.ascontiguousarray(np.asarray(y, dtype=np.float32))
    wk = _hermite_weights(np.asarray(x, dtype=np.float64))  # [4, N_OUT]
    idx = [
        np.asarray(a, dtype=np.int64)
        for a in (y_m1_idx, y0_idx, y1_idx, y2_idx)
    ]
    in_maps = []
    for ci in range(N_CORES):
        j0, j1 = _STARTS[ci], _STARTS[ci + 1]
        n = j1 - j0
        u = np.zeros((4, N_CH, JPAD), np.float32)
        w = np.zeros((4, JPAD), np.float32)
        for t in range(4):
            u[t, :, :n] = y[:, idx[t][j0:j1]]
            w[t, :n] = wk[t, j0:j1]
        # [4, ch, p*F] -> tiles u[t, g, p, c2*F]
        u = u.reshape(4, NG, 2, P, F).transpose(0, 1, 3, 2, 4)
        u = np.ascontiguousarray(u.reshape(4, NG, P, 2 * F))
        w2 = np.repeat(w.reshape(4, P, 1, F), 2, axis=2).reshape(4, P, 2 * F)
        in_maps.append({"u": u, "w": np.ascontiguousarray(w2)})
    return in_maps


def _assemble(results):
    out = np.empty((N_CH, N_OUT), np.float32)
    for ci, res in enumerate(results):
        j0, j1 = _STARTS[ci], _STARTS[ci + 1]
        n = j1 - j0
        o = res["o"].reshape(NG, P, 2, F).transpose(0, 2, 1, 3)
        o = o.reshape(N_CH, JPAD)
        out[:, j0:j1] = o[:, :n]
    return out


def run_on_device(in_maps, trace=False):
    from concourse import bass_utils

    nc = _get_nc()
    return bass_utils.run_bass_kernel_spmd(
        nc, in_maps, core_ids=list(range(N_CORES)), trace=trace
    )


def kernel(y, x, y_m1_idx, y0_idx, y1_idx, y2_idx):
    in_maps = _prep_inputs(y, x, y_m1_idx, y0_idx, y1_idx, y2_idx)
    r = run_on_device(in_maps, trace=False)
    return _assemble(r.results)


if __name__ == "__main__":
    rng = np.random.default_rng(0)
    y = rng.standard_normal((N_CH, N_IN), dtype=np.float32)
    scaling = (N_IN - 1) / (N_OUT - 1) + 1e-12
    xf = np.arange(N_OUT, dtype=np.float64) * scaling
    y0 = np.floor(xf).astype(np.int64)
    y1 = np.clip(y0 + 1, 0, N_IN - 1)
    xv = np.clip(xf - y0, 0.0, 1.0)
    xv[0] = 0.0
    xv[-1] = np.round(xv[-1])
    ym1 = np.clip(y0 - 1, 0, N_IN - 1)
    y2 = np.clip(y1 + 1, 0, N_IN - 1)
    out = kernel(
        y,
        xv.astype(np.float32),
        ym1.astype(np.int32),
        y0.astype(np.int32),
        y1.astype(np.int32),
        y2.astype(np.int32),
    )
    # numpy reference
    c1 = 0.5 * (y[:, y1] - y[:, ym1])
    c2 = y[:, ym1] - 2.5 * y[:, y0] + 2.0 * y[:, y1] - 0.5 * y[:, y2]
    c3 = 0.5 * (y[:, y2] - y[:, ym1]) + 1.5 * (y[:, y0] - y[:, y1])
    xf32 = xv.astype(np.float32)
    exp = ((c3 * xf32 + c2) * xf32 + c1) * xf32 + y[:, y0]
    err = np.abs(out - exp) / np.maximum(np.abs(exp), 1e-3)
    print("self-test max scaled err:", err.max())



# revision 2
# speedup vs baseline: 27387.5946x; 27387.5946x over previous
"""Bass/Trainium2 kernel for nn_Inplace4pHermiteResampler (stencil form).

The resampler indices have rigid structure: y0_idx is strictly increasing
with steps in {1,2} (48k->44.1k ratio ~1.088), and ym1/y1/y2 are
clip(y0-1)/clip(y0+1)/clip(y0+2).  Instead of shipping 4 gathered tap
streams to the device (the naive formulation, ~81 MB/core of DMA), recast
the computation on the INPUT grid:

    out_pad[i] = sum_t W_t[i] * y[i+t-1],   W_t[y0[j]] = w_t(x[j])

a 4-tap stencil with host-scattered FIR weights (W_t[i] = 0 for i not in
the y0 image; those slots are never read).  The true output is the host
compaction out[:, j] = out_pad[:, y0[j]] (y0 injective).  Taps become
shifted SBUF reads (free AP offsets) — no gather anywhere on device.

Device budget per core (measured on TRN2):
  - DVE f32 tensor_tensor is SBUF-bandwidth-bound at ~0.84 ns/elem/lane;
    GPSIMD shares those ports (splitting work across engines serializes),
    so all element-wise work stays on DVE.
  - The 3 accumulate adds are split: device computes TWO partial sums
    (6 DVE ops instead of 7+) and streams both to HBM; the final add runs
    on the host.  This trades cheap DMA (~52 MB/core, overlapped) for a
    1.3x cut in DVE time.  Measured body: ~148 us/core vs 194 us for the
    single-store 7-op version and ~250 us for the gather baseline.

Sharding: core ci owns input block i in [ci*131072, (ci+1)*131072);
index/x prep is replicated host-side, zero device communication.
"""
import os

os.environ.setdefault("NEURON_RT_VIRTUAL_CORE_SIZE", "1")

import numpy as np

N_CH = 32
N_IN = 1_048_576
N_OUT = 963_380
N_CORES = 8
B = N_IN // N_CORES          # 131072 input samples per core
P = 128
F = B // P                   # 1024 free-dim cols per partition row
NG = N_CH // 2               # 16 two-channel groups
HALO = 3
WL = B + HALO                # 131075 window elems per channel
WLP = 131088                 # padded to 16-elem (64B) multiple

UP_BUFS = 4
ACC_BUFS = 3
Q_BUFS = 3


def _hermite_weights(x64: np.ndarray) -> np.ndarray:
    """4 Niemitalo weights per output sample, [4, n] float32."""
    x2 = x64 * x64
    x3 = x2 * x64
    return np.stack(
        [
            -0.5 * x3 + x2 - 0.5 * x64,
            1.5 * x3 - 2.5 * x2 + 1.0,
            -1.5 * x3 + 2.0 * x2 + 0.5 * x64,
            0.5 * x3 - 0.5 * x2,
        ],
        0,
    ).astype(np.float32)


def _build_device_kernel():
    import concourse.bacc as bacc
    import concourse.mybir as mybir
    import concourse.tile as tile
    from concourse.ap import AP

    nc = bacc.Bacc(
        "TRN2",
        target_bir_lowering=False,
        debug=False,
        enable_asserts=False,
        num_devices=N_CORES,
    )
    dt = mybir.dt.float32
    y_d = nc.dram_tensor("yw", [N_CH, WLP], dt, kind="ExternalInput").ap()
    w_d = nc.dram_tensor("w", [4, P, F], dt, kind="ExternalInput").ap()
    o_d = nc.dram_tensor("o", [NG, P, 2, F], dt, kind="ExternalOutput").ap()
    o2_d = nc.dram_tensor("o2", [NG, P, 2, F], dt, kind="ExternalOutput").ap()

    mult = mybir.AluOpType.mult
    add = mybir.AluOpType.add

    with tile.TileContext(nc) as tc:
        with (
            tc.tile_pool(name="wp", bufs=1) as wp,
            tc.tile_pool(name="up", bufs=UP_BUFS) as up,
            tc.tile_pool(name="sp", bufs=ACC_BUFS) as spool,
            tc.tile_pool(name="qp", bufs=Q_BUFS) as qp,
        ):
            wt = []
            for t in range(4):
                w_tile = wp.tile([P, F], dt, tag=f"w{t}")
                eng = nc.sync if t % 2 == 0 else nc.scalar
                eng.dma_start(out=w_tile[:], in_=w_d[t])
                wt.append(w_tile[:].unsqueeze(1).broadcast_to([P, 2, F]))
            for g in range(NG):
                yt = up.tile([P, 2, F + HALO], dt, tag="y")
                src = AP(y_d.tensor, 2 * g * WLP,
                         [(F, P), (WLP, 2), (1, F + HALO)])
                ldeng = nc.sync if g % 2 == 0 else nc.scalar
                ldeng.dma_start(out=yt[:], in_=src)
                # two partial sums; final add happens on the host
                for half, od in ((0, o_d), (1, o2_d)):
                    s = spool.tile([P, 2, F], dt, tag=f"s{half}")
                    q = qp.tile([P, 2, F], dt, tag="q")
                    t0, t1 = 2 * half, 2 * half + 1
                    nc.vector.tensor_tensor(
                        out=s[:], in0=yt[:, :, t0:t0 + F], in1=wt[t0], op=mult)
                    nc.vector.tensor_tensor(
                        out=q[:], in0=yt[:, :, t1:t1 + F], in1=wt[t1], op=mult)
                    nc.vector.tensor_tensor(
                        out=s[:], in0=s[:], in1=q[:], op=add)
                    (nc.sync if half == 0 else nc.scalar).dma_start(
                        out=od[g], in_=s[:])
    nc.compile()
    return nc


_NC_CACHE = None


def _get_nc():
    global _NC_CACHE
    if _NC_CACHE is None:
        _NC_CACHE = _build_device_kernel()
    return _NC_CACHE


def _check_structure(y0, ym1, y1, y2):
    d = np.diff(y0)
    if d.size == 0 or not (d.min() >= 1 and d.max() <= 2):
        return False
    if not np.array_equal(ym1, np.maximum(y0 - 1, 0)):
        return False
    if not np.array_equal(y1, np.minimum(y0 + 1, N_IN - 1)):
        return False
    return np.array_equal(y2, np.minimum(y1 + 1, N_IN - 1))


def _prep_inputs(y, x, y_m1_idx, y0_idx, y1_idx, y2_idx):
    """Host-side restructure. Returns (in_maps, y0), or None when the
    indices don't match the resampler pattern (caller falls back)."""
    y = np.ascontiguousarray(np.asarray(y, dtype=np.float32))
    y0 = np.asarray(y0_idx, dtype=np.int64)
    if y.shape != (N_CH, N_IN) or y0.shape != (N_OUT,):
        return None
    if not _check_structure(
        y0,
        np.asarray(y_m1_idx, dtype=np.int64),
        np.asarray(y1_idx, dtype=np.int64),
        np.asarray(y2_idx, dtype=np.int64),
    ):
        return None
    wk = _hermite_weights(np.asarray(x, dtype=np.float64))  # [4, N_OUT]
    # scatter weights onto the input grid (y0 strictly increasing)
    W = np.zeros((4, N_IN), np.float32)
    W[:, y0] = wk
    # edge-replicated input for halo taps
    ypad = np.pad(y, ((0, 0), (1, 2)), mode="edge")  # [32, N_IN+3]
    in_maps = []
    for ci in range(N_CORES):
        i0 = B * ci
        yw = np.zeros((N_CH, WLP), np.float32)
        yw[:, :WL] = ypad[:, i0:i0 + WL]
        Wl = np.ascontiguousarray(W[:, i0:i0 + B].reshape(4, P, F))
        in_maps.append({"yw": yw, "w": Wl})
    return in_maps, y0


def _assemble(results, y0):
    op = np.empty((N_CH, N_IN), np.float32)
    for ci, res in enumerate(results):
        o = res["o"] + res["o2"]
        o = o.reshape(NG, P, 2, F).transpose(0, 2, 1, 3)
        op[:, B * ci:B * (ci + 1)] = o.reshape(N_CH, B)
    return np.ascontiguousarray(op[:, y0])


def run_on_device(in_maps, trace=False):
    from concourse import bass_utils

    nc = _get_nc()
    return bass_utils.run_bass_kernel_spmd(
        nc, in_maps, core_ids=list(range(N_CORES)), trace=trace
    )


def _fallback(y, x, y_m1_idx, y0_idx, y1_idx, y2_idx):
    """Generic-index path (never hit for the real resampler inputs)."""
    y = np.asarray(y, np.float32)
    x = np.asarray(x, np.float32)
    ym1 = y[:, np.asarray(y_m1_idx, np.int64)]
    y0v = y[:, np.asarray(y0_idx, np.int64)]
    y1v = y[:, np.asarray(y1_idx, np.int64)]
    y2v = y[:, np.asarray(y2_idx, np.int64)]
    c1 = np.float32(0.5) * (y1v - ym1)
    c2 = ym1 - np.float32(2.5) * y0v + np.float32(2.0) * y1v \
        - np.float32(0.5) * y2v
    c3 = np.float32(0.5) * (y2v - ym1) + np.float32(1.5) * (y0v - y1v)
    return ((c3 * x + c2) * x + c1) * x + y0v


def kernel(y, x, y_m1_idx, y0_idx, y1_idx, y2_idx):
    prep = _prep_inputs(y, x, y_m1_idx, y0_idx, y1_idx, y2_idx)
    if prep is None:
        return _fallback(y, x, y_m1_idx, y0_idx, y1_idx, y2_idx)
    in_maps, y0 = prep
    r = run_on_device(in_maps, trace=False)
    return _assemble(r.results, y0)


if __name__ == "__main__":
    rng = np.random.default_rng(0)
    y = rng.standard_normal((N_CH, N_IN), dtype=np.float32)
    scaling = (N_IN - 1) / (N_OUT - 1) + 1e-12
    xf = np.arange(N_OUT, dtype=np.float64) * scaling
    y0 = np.floor(xf).astype(np.int64)
    y1 = np.clip(y0 + 1, 0, N_IN - 1)
    xv = np.clip(xf - y0, 0.0, 1.0)
    xv[0] = 0.0
    xv[-1] = np.round(xv[-1])
    ym1 = np.clip(y0 - 1, 0, N_IN - 1)
    y2 = np.clip(y1 + 1, 0, N_IN - 1)
    out = kernel(
        y,
        xv.astype(np.float32),
        ym1.astype(np.int32),
        y0.astype(np.int32),
        y1.astype(np.int32),
        y2.astype(np.int32),
    )
    c1 = 0.5 * (y[:, y1] - y[:, ym1])
    c2 = y[:, ym1] - 2.5 * y[:, y0] + 2.0 * y[:, y1] - 0.5 * y[:, y2]
    c3 = 0.5 * (y[:, y2] - y[:, ym1]) + 1.5 * (y[:, y0] - y[:, y1])
    xf32 = xv.astype(np.float32)
    exp = ((c3 * xf32 + c2) * xf32 + c1) * xf32 + y[:, y0]
    err = np.abs(out - exp) / np.maximum(np.abs(exp), 1e-3)
    print("self-test max scaled err:", err.max())


# revision 3
# speedup vs baseline: 132621.1027x; 4.8424x over previous
"""Bass/Trainium2 kernel for nn_Inplace4pHermiteResampler (stencil form).

The resampler indices have rigid structure: y0_idx is strictly increasing
with steps in {1,2} (48k->44.1k ratio ~1.088), and ym1/y1/y2 are
clip(y0-1)/clip(y0+1)/clip(y0+2).  Instead of shipping 4 gathered tap
streams to the device (the naive formulation, ~81 MB/core of DMA), recast
the computation on the INPUT grid:

    out_pad[i] = sum_t W_t[i] * y[i+t-1],   W_t[y0[j]] = w_t(x[j])

a 4-tap stencil with host-scattered FIR weights (W_t[i] = 0 for i not in
the y0 image; those slots are never read).  The true output is the host
compaction out[:, j] = out_pad[:, y0[j]] (y0 injective).  Taps become
shifted SBUF reads (free AP offsets) — no gather anywhere on device.

Device budget per core (measured on TRN2):
  - DVE f32 tensor_tensor is SBUF-bandwidth-bound at ~0.84 ns/elem/lane;
    GPSIMD shares those ports (splitting work across engines serializes),
    so all element-wise work stays on DVE.
  - The 3 accumulate adds are split: device computes TWO partial sums
    (6 DVE ops instead of 7+) and streams both to HBM; the final add runs
    on the host.  This trades cheap DMA (~52 MB/core, overlapped) for a
    1.3x cut in DVE time.  Measured body: ~148 us/core vs 194 us for the
    single-store 7-op version and ~250 us for the gather baseline.

Sharding: core ci owns input block i in [ci*131072, (ci+1)*131072);
index/x prep is replicated host-side, zero device communication.
"""
import os

os.environ.setdefault("NEURON_RT_VIRTUAL_CORE_SIZE", "1")

import numpy as np

N_CH = 32
N_IN = 1_048_576
N_OUT = 963_380
N_CORES = 8
B = N_IN // N_CORES          # 131072 input samples per core
P = 128
F = B // P                   # 1024 free-dim cols per partition row
NG = N_CH // 2               # 16 two-channel groups
HALO = 3
WL = B + HALO                # 131075 window elems per channel
WLP = 131088                 # padded to 16-elem (64B) multiple

UP_BUFS = 4
ACC_BUFS = 3
Q_BUFS = 3


def _hermite_weights(x64: np.ndarray) -> np.ndarray:
    """4 Niemitalo weights per output sample, [4, n] float32."""
    x2 = x64 * x64
    x3 = x2 * x64
    return np.stack(
        [
            -0.5 * x3 + x2 - 0.5 * x64,
            1.5 * x3 - 2.5 * x2 + 1.0,
            -1.5 * x3 + 2.0 * x2 + 0.5 * x64,
            0.5 * x3 - 0.5 * x2,
        ],
        0,
    ).astype(np.float32)


def _build_device_kernel():
    import concourse.bacc as bacc
    import concourse.mybir as mybir
    import concourse.tile as tile
    from concourse.ap import AP

    nc = bacc.Bacc(
        "TRN2",
        target_bir_lowering=False,
        debug=False,
        enable_asserts=False,
        num_devices=N_CORES,
    )
    dt = mybir.dt.float32
    y_d = nc.dram_tensor("yw", [N_CH, WLP], dt, kind="ExternalInput").ap()
    w_d = nc.dram_tensor("w", [4, P, F], dt, kind="ExternalInput").ap()
    o_d = nc.dram_tensor("o", [NG, P, 2, F], dt, kind="ExternalOutput").ap()
    o2_d = nc.dram_tensor("o2", [NG, P, 2, F], dt, kind="ExternalOutput").ap()

    mult = mybir.AluOpType.mult
    add = mybir.AluOpType.add

    with tile.TileContext(nc) as tc:
        with (
            tc.tile_pool(name="wp", bufs=1) as wp,
            tc.tile_pool(name="up", bufs=UP_BUFS) as up,
            tc.tile_pool(name="sp", bufs=ACC_BUFS) as spool,
            tc.tile_pool(name="qp", bufs=Q_BUFS) as qp,
        ):
            # Ramp: group 0 needs w0..w3 + its y tile before compute starts.
            # Issue order puts {w0,w1} (s0's weights) on ACT and y0 first on
            # SP, so the first partial-sum chain starts after ~1.6 MB/queue
            # instead of waiting for all four weight tiles.
            wt = []
            w_tiles = []
            for t in range(4):
                w_tile = wp.tile([P, F], dt, tag=f"w{t}")
                w_tiles.append(w_tile)
                wt.append(w_tile[:].unsqueeze(1).broadcast_to([P, 2, F]))
            nc.scalar.dma_start(out=w_tiles[0][:], in_=w_d[0])
            nc.scalar.dma_start(out=w_tiles[1][:], in_=w_d[1])
            for g in range(NG):
                yt = up.tile([P, 2, F + HALO], dt, tag="y")
                src = AP(y_d.tensor, 2 * g * WLP,
                         [(F, P), (WLP, 2), (1, F + HALO)])
                ldeng = nc.sync if g % 2 == 0 else nc.scalar
                ldeng.dma_start(out=yt[:], in_=src)
                if g == 0:
                    nc.sync.dma_start(out=w_tiles[2][:], in_=w_d[2])
                    nc.scalar.dma_start(out=w_tiles[3][:], in_=w_d[3])
                # two partial sums; final add happens on the host
                for half, od in ((0, o_d), (1, o2_d)):
                    s = spool.tile([P, 2, F], dt, tag=f"s{half}")
                    q = qp.tile([P, 2, F], dt, tag="q")
                    t0, t1 = 2 * half, 2 * half + 1
                    nc.vector.tensor_tensor(
                        out=s[:], in0=yt[:, :, t0:t0 + F], in1=wt[t0], op=mult)
                    nc.vector.tensor_tensor(
                        out=q[:], in0=yt[:, :, t1:t1 + F], in1=wt[t1], op=mult)
                    nc.vector.tensor_tensor(
                        out=s[:], in0=s[:], in1=q[:], op=add)
                    (nc.sync if half == 0 else nc.scalar).dma_start(
                        out=od[g], in_=s[:])
    nc.compile()
    return nc


_NC_CACHE = None


def _get_nc():
    global _NC_CACHE
    if _NC_CACHE is None:
        _NC_CACHE = _build_device_kernel()
    return _NC_CACHE


def _check_structure(y0, ym1, y1, y2):
    d = np.diff(y0)
    if d.size == 0 or not (d.min() >= 1 and d.max() <= 2):
        return False
    if not np.array_equal(ym1, np.maximum(y0 - 1, 0)):
        return False
    if not np.array_equal(y1, np.minimum(y0 + 1, N_IN - 1)):
        return False
    return np.array_equal(y2, np.minimum(y1 + 1, N_IN - 1))


def _prep_inputs(y, x, y_m1_idx, y0_idx, y1_idx, y2_idx):
    """Host-side restructure. Returns (in_maps, y0), or None when the
    indices don't match the resampler pattern (caller falls back)."""
    y = np.ascontiguousarray(np.asarray(y, dtype=np.float32))
    y0 = np.asarray(y0_idx, dtype=np.int64)
    if y.shape != (N_CH, N_IN) or y0.shape != (N_OUT,):
        return None
    if not _check_structure(
        y0,
        np.asarray(y_m1_idx, dtype=np.int64),
        np.asarray(y1_idx, dtype=np.int64),
        np.asarray(y2_idx, dtype=np.int64),
    ):
        return None
    wk = _hermite_weights(np.asarray(x, dtype=np.float64))  # [4, N_OUT]
    # scatter weights onto the input grid (y0 strictly increasing)
    W = np.zeros((4, N_IN), np.float32)
    W[:, y0] = wk
    # edge-replicated input for halo taps
    ypad = np.pad(y, ((0, 0), (1, 2)), mode="edge")  # [32, N_IN+3]
    in_maps = []
    for ci in range(N_CORES):
        i0 = B * ci
        yw = np.zeros((N_CH, WLP), np.float32)
        yw[:, :WL] = ypad[:, i0:i0 + WL]
        Wl = np.ascontiguousarray(W[:, i0:i0 + B].reshape(4, P, F))
        in_maps.append({"yw": yw, "w": Wl})
    return in_maps, y0


def _assemble(results, y0):
    op = np.empty((N_CH, N_IN), np.float32)
    for ci, res in enumerate(results):
        o = res["o"] + res["o2"]
        o = o.reshape(NG, P, 2, F).transpose(0, 2, 1, 3)
        op[:, B * ci:B * (ci + 1)] = o.reshape(N_CH, B)
    return np.ascontiguousarray(op[:, y0])


def run_on_device(in_maps, trace=False):
    from concourse import bass_utils

    nc = _get_nc()
    return bass_utils.run_bass_kernel_spmd(
        nc, in_maps, core_ids=list(range(N_CORES)), trace=trace
    )


def _fallback(y, x, y_m1_idx, y0_idx, y1_idx, y2_idx):
    """Generic-index path (never hit for the real resampler inputs)."""
    y = np.asarray(y, np.float32)
    x = np.asarray(x, np.float32)
    ym1 = y[:, np.asarray(y_m1_idx, np.int64)]
    y0v = y[:, np.asarray(y0_idx, np.int64)]
    y1v = y[:, np.asarray(y1_idx, np.int64)]
    y2v = y[:, np.asarray(y2_idx, np.int64)]
    c1 = np.float32(0.5) * (y1v - ym1)
    c2 = ym1 - np.float32(2.5) * y0v + np.float32(2.0) * y1v \
        - np.float32(0.5) * y2v
    c3 = np.float32(0.5) * (y2v - ym1) + np.float32(1.5) * (y0v - y1v)
    return ((c3 * x + c2) * x + c1) * x + y0v


def kernel(y, x, y_m1_idx, y0_idx, y1_idx, y2_idx):
    prep = _prep_inputs(y, x, y_m1_idx, y0_idx, y1_idx, y2_idx)
    if prep is None:
        return _fallback(y, x, y_m1_idx, y0_idx, y1_idx, y2_idx)
    in_maps, y0 = prep
    r = run_on_device(in_maps, trace=False)
    return _assemble(r.results, y0)


if __name__ == "__main__":
    rng = np.random.default_rng(0)
    y = rng.standard_normal((N_CH, N_IN), dtype=np.float32)
    scaling = (N_IN - 1) / (N_OUT - 1) + 1e-12
    xf = np.arange(N_OUT, dtype=np.float64) * scaling
    y0 = np.floor(xf).astype(np.int64)
    y1 = np.clip(y0 + 1, 0, N_IN - 1)
    xv = np.clip(xf - y0, 0.0, 1.0)
    xv[0] = 0.0
    xv[-1] = np.round(xv[-1])
    ym1 = np.clip(y0 - 1, 0, N_IN - 1)
    y2 = np.clip(y1 + 1, 0, N_IN - 1)
    out = kernel(
        y,
        xv.astype(np.float32),
        ym1.astype(np.int32),
        y0.astype(np.int32),
        y1.astype(np.int32),
        y2.astype(np.int32),
    )
    c1 = 0.5 * (y[:, y1] - y[:, ym1])
    c2 = y[:, ym1] - 2.5 * y[:, y0] + 2.0 * y[:, y1] - 0.5 * y[:, y2]
    c3 = 0.5 * (y[:, y2] - y[:, ym1]) + 1.5 * (y[:, y0] - y[:, y1])
    xf32 = xv.astype(np.float32)
    exp = ((c3 * xf32 + c2) * xf32 + c1) * xf32 + y[:, y0]
    err = np.abs(out - exp) / np.maximum(np.abs(exp), 1e-3)
    print("self-test max scaled err:", err.max())
